# revision 7
# baseline (speedup 1.0000x reference)
"""Per-sample adaptive 3x3 conv (B=8, T=4, 256->256ch, 64x64, pad=1) on 8 TRN2 cores.

Sharding: sample b -> core b. Each core convolves its own [T=4,256,64,64]
block with its own [256,256,3,3] weight; no collectives.

Device kernel (per core): shift-and-accumulate conv-as-matmul.
  - input stored zero-padded in SBUF as [128 ci, 66*66] bf16 (2 Cin chunks, 4 images)
  - weights as 36 lhsT tiles [128 ci, 128 co] bf16 (9 offsets x 2 cin x 2 cout chunks)
  - per output tile [128 co, 8 rows * 64 cols = 512]: accumulate 18 matmuls into
    one PSUM bank, DVE-copy to SBUF, DMA to DRAM.
"""

import numpy as np
import ml_dtypes

B, T, CIN, COUT, H, W = 8, 4, 256, 256, 64, 64
HP, WP = H + 2, W + 2  # zero-padded spatial
KH, KW = 3, 3
NCORES = 8
CH = 2           # channel chunks of 128
ROWS_PER_TILE = 8  # output rows per matmul group -> N = 8*64 = 512
NYT = H // ROWS_PER_TILE

_cache = {}

# Populated by the most recent profiled run (see kernel(..., profile=True)).
LAST_EXEC_TIME_NS = None
LAST_PROFILE = None


def _build():
    import concourse.mybir as mybir
    import concourse.tile as tile
    from concourse import bacc

    nc = bacc.Bacc(
        "TRN2",
        target_bir_lowering=False,
        debug=False,
        enable_asserts=False,
        num_devices=NCORES,
    )
    x_d = nc.dram_tensor(
        "x", [T, CH, 128, HP * WP], mybir.dt.bfloat16, kind="ExternalInput"
    ).ap()
    w_d = nc.dram_tensor(
        "w", [128, KH * KW * CH * CH * 128], mybir.dt.bfloat16, kind="ExternalInput"
    ).ap()
    o_d = nc.dram_tensor(
        "out", [T, CH, 128, H * W], mybir.dt.float32, kind="ExternalOutput"
    ).ap()

    # input rows are DMA'd in 3 row-blocks so compute can start after the
    # first block of image 0 lands (Tile deps are region-based)
    ROW_BLOCKS = [(0, 22), (22, 44), (44, 66)]

    with tile.TileContext(nc) as tc:
        with (
            tc.tile_pool(name="persist", bufs=1) as persist,
            tc.tile_pool(name="psum", bufs=8, space="PSUM") as psum_pool,
            tc.tile_pool(name="obuf", bufs=4) as out_pool,
        ):
            w_sb = persist.tile([128, KH * KW * CH * CH * 128], mybir.dt.bfloat16, tag="w")
            x_sb = {}
            for t in range(T):
                for c in range(CH):
                    x_sb[(t, c)] = persist.tile(
                        [128, HP * WP],
                        mybir.dt.bfloat16,
                        name=f"x{t}{c}",
                        tag=f"x{t}{c}",
                    )

            NINE = KH * KW * 128  # one (coc, cic) group of 9 weight tiles
            # first-needed slivers first: 9 tiles of (coc0,cic0), x0 rows 0-10,
            # then the rest in need order
            nc.sync.dma_start(w_sb[:, :NINE], w_d[:, :NINE])
            nc.sync.dma_start(x_sb[(0, 0)][:, : 11 * WP], x_d[0, 0, :, : 11 * WP])
            nc.sync.dma_start(w_sb[:, NINE : 2 * NINE], w_d[:, NINE : 2 * NINE])
            nc.sync.dma_start(x_sb[(0, 1)][:, : 11 * WP], x_d[0, 1, :, : 11 * WP])
            nc.sync.dma_start(w_sb[:, 2 * NINE :], w_d[:, 2 * NINE :])
            for c in range(CH):
                nc.sync.dma_start(
                    x_sb[(0, c)][:, 11 * WP : 22 * WP], x_d[0, c, :, 11 * WP : 22 * WP]
                )
            for t in range(T):
                for r0, r1 in ROW_BLOCKS:
                    if t == 0 and r0 == 0:
                        continue
                    for c in range(CH):
                        nc.sync.dma_start(
                            x_sb[(t, c)][:, r0 * WP : r1 * WP],
                            x_d[t, c, :, r0 * WP : r1 * WP],
                        )

            NK = CH * KH * KW  # 18 accumulation steps per output tile
            for t in range(T):
                x3 = {
                    c: x_sb[(t, c)][:].rearrange("p (h w) -> p h w", w=WP)
                    for c in range(CH)
                }
                for coc in range(CH):
                    for ytp in range(NYT // 2):
                        # pair of row-tiles sharing each weight load; the very
                        # last row-tile is split in half so its evacuation
                        # overlaps the end of the matmul stream
                        last = t == T - 1 and coc == CH - 1 and ytp == NYT // 2 - 1
                        subtiles = []  # (psum_tile, y0, nrows)
                        ps0 = psum_pool.tile(
                            [128, ROWS_PER_TILE * W], mybir.dt.float32, bufs=4
                        )
                        subtiles.append((ps0, 2 * ytp * ROWS_PER_TILE, ROWS_PER_TILE))
                        y1 = (2 * ytp + 1) * ROWS_PER_TILE
                        if not last:
                            ps1 = psum_pool.tile(
                                [128, ROWS_PER_TILE * W],
                                mybir.dt.float32,
                                bufs=4,
                                tag="ps1",
                            )
                            subtiles.append((ps1, y1, ROWS_PER_TILE))
                        else:
                            half = ROWS_PER_TILE // 2
                            for j in range(2):
                                psh = psum_pool.tile(
                                    [128, half * W],
                                    mybir.dt.float32,
                                    bufs=4,
                                    tag="ps1",
                                    name=f"ps1h{j}",
                                )
                                subtiles.append((psh, y1 + j * half, half))
                        for k in range(NK):
                            cic, ky, kx = k // 9, (k % 9) // KW, k % KW
                            idx = (coc * CH + cic) * KH * KW + ky * KW + kx
                            w_ap = w_sb[:, idx * 128 : (idx + 1) * 128]
                            for ps, y0, nrows in subtiles:
                                nc.tensor.matmul(
                                    ps[:],
                                    w_ap,
                                    x3[cic][:, y0 + ky : y0 + ky + nrows, kx : kx + W],
                                    start=(k == 0),
                                    stop=(k == NK - 1),
                                )
                        for ps, y0, nrows in subtiles:
                            ob = out_pool.tile([128, nrows * W], mybir.dt.float32)
                            nc.vector.tensor_copy(ob[:], ps[:])
                            nc.scalar.dma_start(
                                o_d[t, coc, :, y0 * W : (y0 + nrows) * W], ob[:]
                            )

    nc.compile()
    return nc


def _prep_inputs(inputs, ada_weight):
    """Host-side layout prep: pad+cast inputs, transpose weights to lhsT tiles."""
    bf16 = ml_dtypes.bfloat16
    in_maps = []
    for b in range(B):
        xb = inputs[b * T : (b + 1) * T].reshape(T, CH, 128, H, W)
        xp = np.zeros((T, CH, 128, HP, WP), dtype=bf16)
        xp[..., 1 : H + 1, 1 : W + 1] = xb.astype(bf16)

        # ada_weight[b]: [co, ci, ky, kx] -> lhsT tiles [ci, (coc cic ky kx co)]
        wb = ada_weight[b].reshape(CH, 128, CH, 128, KH, KW)  # coc co cic ci ky kx
        wprep = wb.transpose(3, 0, 2, 4, 5, 1)  # ci coc cic ky kx co
        wprep = np.ascontiguousarray(wprep.astype(bf16)).reshape(
            128, KH * KW * CH * CH * 128
        )
        in_maps.append({"x": xp.reshape(T, CH, 128, HP * WP), "w": wprep})
    return in_maps


def _setup_profiling():
    """Register the NTFF profile hook that the image's antenv stub lacks,
    and keep profiling artifacts local. Only used when profile=True."""
    import sys
    import types

    try:
        from antenv.axon_hooks import get_axon_ntff_profile_hook  # noqa: F401

        return
    except ImportError:
        pass
    import antenv
    from trn_agent_boot.trn_boot import _ntff_profile_via_ctypes

    hook = _ntff_profile_via_ctypes("/opt/axon/libaxon_pjrt.so")
    m = types.ModuleType("antenv.axon_hooks")
    m.get_axon_ntff_profile_hook = lambda: hook
    m.set_axon_ntff_profile_hook = lambda h: None
    sys.modules["antenv.axon_hooks"] = m
    antenv.axon_hooks = m

    from concourse import bass_utils

    bass_utils.upload_artifacts = lambda tmpdir: f"file://{tmpdir}"


def kernel(inputs, ada_weight, profile=False, trace_kwargs=None):
    global LAST_EXEC_TIME_NS, LAST_PROFILE
    from concourse.bass_utils import run_bass_kernel_spmd

    if profile:
        _setup_profiling()
    if "nc" not in _cache:
        _cache["nc"] = _build()
    nc = _cache["nc"]

    in_maps = _prep_inputs(np.asarray(inputs), np.asarray(ada_weight))

    kwargs = {}
    if profile:
        kwargs["trace"] = True
        if trace_kwargs:
            kwargs.update(trace_kwargs)
    res = run_bass_kernel_spmd(nc, in_maps, core_ids=list(range(NCORES)), **kwargs)
    if profile:
        LAST_EXEC_TIME_NS = res.exec_time_ns
        LAST_PROFILE = res

    out = np.stack([res.results[b]["out"].reshape(T, COUT, H, W) for b in range(B)])
    return np.ascontiguousarray(out.reshape(B * T, COUT, H, W).astype(np.float32))


# revision 8
# speedup vs baseline: 1.2017x; 1.2017x over previous
"""Per-sample adaptive 3x3 conv (B=8, T=4, 256->256ch, 64x64, pad=1) on 8 TRN2 cores.

Sharding: sample b -> core b. Each core convolves its own [T=4,256,64,64]
block with its own [256,256,3,3] weight; no collectives.

Device kernel (per core): shift-and-accumulate conv-as-matmul.
  - input stored zero-padded in SBUF as [128 ci, 66*66] bf16 (2 Cin chunks, 4 images)
  - weights as 36 lhsT tiles [128 ci, 128 co] bf16 (9 offsets x 2 cin x 2 cout chunks)
  - per output tile [128 co, 8 rows * 64 cols = 512]: accumulate 18 matmuls into
    one PSUM bank, DVE-copy to SBUF, DMA to DRAM.
"""

import numpy as np
import ml_dtypes

B, T, CIN, COUT, H, W = 8, 4, 256, 256, 64, 64
HP, WP = H + 2, W + 2  # zero-padded spatial
KH, KW = 3, 3
NCORES = 8
CH = 2           # channel chunks of 128
ROWS_PER_TILE = 8  # output rows per matmul group -> N = 8*64 = 512
NYT = H // ROWS_PER_TILE

_cache = {}

# Populated by the most recent profiled run (see kernel(..., profile=True)).
LAST_EXEC_TIME_NS = None
LAST_PROFILE = None


def _build():
    import concourse.mybir as mybir
    import concourse.tile as tile
    from concourse import bacc

    nc = bacc.Bacc(
        "TRN2",
        target_bir_lowering=False,
        debug=False,
        enable_asserts=False,
        num_devices=NCORES,
    )
    x_d = nc.dram_tensor(
        "x", [T, CH, 128, HP * WP], mybir.dt.bfloat16, kind="ExternalInput"
    ).ap()
    w_d = nc.dram_tensor(
        "w", [128, KH * KW * CH * CH * 128], mybir.dt.bfloat16, kind="ExternalInput"
    ).ap()
    o_d = nc.dram_tensor(
        "out", [T, CH, 128, H * W], mybir.dt.float32, kind="ExternalOutput"
    ).ap()

    # input rows are DMA'd in 3 row-blocks so compute can start after the
    # first block of image 0 lands (Tile deps are region-based)
    ROW_BLOCKS = [(0, 22), (22, 44), (44, 66)]

    with tile.TileContext(nc) as tc:
        with (
            tc.tile_pool(name="persist", bufs=1) as persist,
            tc.tile_pool(name="psum", bufs=8, space="PSUM") as psum_pool,
            tc.tile_pool(name="obuf", bufs=4) as out_pool,
        ):
            w_sb = persist.tile([128, KH * KW * CH * CH * 128], mybir.dt.bfloat16, tag="w")
            x_sb = {}
            for t in range(T):
                for c in range(CH):
                    x_sb[(t, c)] = persist.tile(
                        [128, HP * WP],
                        mybir.dt.bfloat16,
                        name=f"x{t}{c}",
                        tag=f"x{t}{c}",
                    )

            NINE = KH * KW * 128  # one (coc, cic) group of 9 weight tiles
            # first-needed slivers first: 9 tiles of (coc0,cic0), x0 rows 0-10,
            # then the rest in need order
            nc.sync.dma_start(w_sb[:, :NINE], w_d[:, :NINE])
            nc.sync.dma_start(x_sb[(0, 0)][:, : 11 * WP], x_d[0, 0, :, : 11 * WP])
            nc.sync.dma_start(w_sb[:, NINE : 2 * NINE], w_d[:, NINE : 2 * NINE])
            nc.sync.dma_start(x_sb[(0, 1)][:, : 11 * WP], x_d[0, 1, :, : 11 * WP])
            nc.sync.dma_start(w_sb[:, 2 * NINE :], w_d[:, 2 * NINE :])
            for c in range(CH):
                nc.sync.dma_start(
                    x_sb[(0, c)][:, 11 * WP : 22 * WP], x_d[0, c, :, 11 * WP : 22 * WP]
                )
            for t in range(T):
                for r0, r1 in ROW_BLOCKS:
                    if t == 0 and r0 == 0:
                        continue
                    for c in range(CH):
                        nc.sync.dma_start(
                            x_sb[(t, c)][:, r0 * WP : r1 * WP],
                            x_d[t, c, :, r0 * WP : r1 * WP],
                        )

            NK = CH * KH * KW  # 18 accumulation steps per output tile
            for t in range(T):
                x3 = {
                    c: x_sb[(t, c)][:].rearrange("p (h w) -> p h w", w=WP)
                    for c in range(CH)
                }
                for coc in range(CH):
                    for yt in range(NYT):
                        ps = psum_pool.tile([128, ROWS_PER_TILE * W], mybir.dt.float32)
                        y0 = yt * ROWS_PER_TILE
                        for k in range(NK):
                            cic, ky, kx = k // 9, (k % 9) // KW, k % KW
                            idx = (coc * CH + cic) * KH * KW + ky * KW + kx
                            nc.tensor.matmul(
                                ps[:],
                                w_sb[:, idx * 128 : (idx + 1) * 128],
                                x3[cic][
                                    :, y0 + ky : y0 + ky + ROWS_PER_TILE, kx : kx + W
                                ],
                                start=(k == 0),
                                stop=(k == NK - 1),
                            )
                        ob = out_pool.tile([128, ROWS_PER_TILE * W], mybir.dt.float32)
                        nc.vector.tensor_copy(ob[:], ps[:])
                        nc.scalar.dma_start(
                            o_d[t, coc, :, y0 * W : (y0 + ROWS_PER_TILE) * W], ob[:]
                        )

    nc.compile()
    return nc


def _prep_inputs(inputs, ada_weight):
    """Host-side layout prep: pad+cast inputs, transpose weights to lhsT tiles."""
    bf16 = ml_dtypes.bfloat16
    in_maps = []
    for b in range(B):
        xb = inputs[b * T : (b + 1) * T].reshape(T, CH, 128, H, W)
        xp = np.zeros((T, CH, 128, HP, WP), dtype=bf16)
        xp[..., 1 : H + 1, 1 : W + 1] = xb.astype(bf16)

        # ada_weight[b]: [co, ci, ky, kx] -> lhsT tiles [ci, (coc cic ky kx co)]
        wb = ada_weight[b].reshape(CH, 128, CH, 128, KH, KW)  # coc co cic ci ky kx
        wprep = wb.transpose(3, 0, 2, 4, 5, 1)  # ci coc cic ky kx co
        wprep = np.ascontiguousarray(wprep.astype(bf16)).reshape(
            128, KH * KW * CH * CH * 128
        )
        in_maps.append({"x": xp.reshape(T, CH, 128, HP * WP), "w": wprep})
    return in_maps


def _setup_profiling():
    """Register the NTFF profile hook that the image's antenv stub lacks,
    and keep profiling artifacts local. Only used when profile=True."""
    import sys
    import types

    try:
        from antenv.axon_hooks import get_axon_ntff_profile_hook  # noqa: F401

        return
    except ImportError:
        pass
    import antenv
    from trn_agent_boot.trn_boot import _ntff_profile_via_ctypes

    hook = _ntff_profile_via_ctypes("/opt/axon/libaxon_pjrt.so")
    m = types.ModuleType("antenv.axon_hooks")
    m.get_axon_ntff_profile_hook = lambda: hook
    m.set_axon_ntff_profile_hook = lambda h: None
    sys.modules["antenv.axon_hooks"] = m
    antenv.axon_hooks = m

    from concourse import bass_utils

    bass_utils.upload_artifacts = lambda tmpdir: f"file://{tmpdir}"


def kernel(inputs, ada_weight, profile=False, trace_kwargs=None):
    global LAST_EXEC_TIME_NS, LAST_PROFILE
    from concourse.bass_utils import run_bass_kernel_spmd

    if profile:
        _setup_profiling()
    if "nc" not in _cache:
        _cache["nc"] = _build()
    nc = _cache["nc"]

    in_maps = _prep_inputs(np.asarray(inputs), np.asarray(ada_weight))

    kwargs = {}
    if profile:
        kwargs["trace"] = True
        if trace_kwargs:
            kwargs.update(trace_kwargs)
    res = run_bass_kernel_spmd(nc, in_maps, core_ids=list(range(NCORES)), **kwargs)
    if profile:
        LAST_EXEC_TIME_NS = res.exec_time_ns
        LAST_PROFILE = res

    out = np.stack([res.results[b]["out"].reshape(T, COUT, H, W) for b in range(B)])
    return np.ascontiguousarray(out.reshape(B * T, COUT, H, W).astype(np.float32))


# revision 10
# speedup vs baseline: 1.2050x; 1.0027x over previous
"""Per-sample adaptive 3x3 conv (B=8, T=4, 256->256ch, 64x64, pad=1) on 8 TRN2 cores.

Sharding: sample b -> core b. Each core convolves its own [T=4,256,64,64]
block with its own [256,256,3,3] weight; no collectives.

Device kernel (per core): shift-and-accumulate conv-as-matmul.
  - input stored zero-padded in SBUF as [128 ci, 66*66] bf16 (2 Cin chunks, 4 images)
  - weights as 36 lhsT tiles [128 ci, 128 co] bf16 (9 offsets x 2 cin x 2 cout chunks)
  - per output tile [128 co, 8 rows * 64 cols = 512]: accumulate 18 matmuls into
    one PSUM bank, DVE-copy to SBUF, DMA to DRAM.
"""

import numpy as np
import ml_dtypes

B, T, CIN, COUT, H, W = 8, 4, 256, 256, 64, 64
HP, WP = H + 2, W + 2  # zero-padded spatial
KH, KW = 3, 3
NCORES = 8
CH = 2           # channel chunks of 128
ROWS_PER_TILE = 8  # output rows per matmul group -> N = 8*64 = 512
NYT = H // ROWS_PER_TILE

_cache = {}

# Populated by the most recent profiled run (see kernel(..., profile=True)).
LAST_EXEC_TIME_NS = None
LAST_PROFILE = None


def _build():
    import concourse.mybir as mybir
    import concourse.tile as tile
    from concourse import bacc

    nc = bacc.Bacc(
        "TRN2",
        target_bir_lowering=False,
        debug=False,
        enable_asserts=False,
        num_devices=NCORES,
    )
    x_d = nc.dram_tensor(
        "x", [T, CH, 128, HP * WP], mybir.dt.bfloat16, kind="ExternalInput"
    ).ap()
    w_d = nc.dram_tensor(
        "w", [128, KH * KW * CH * CH * 128], mybir.dt.bfloat16, kind="ExternalInput"
    ).ap()
    o_d = nc.dram_tensor(
        "out", [T, CH, 128, H * W], mybir.dt.float32, kind="ExternalOutput"
    ).ap()

    # input rows are DMA'd in 3 row-blocks so compute can start after the
    # first block of image 0 lands (Tile deps are region-based)
    ROW_BLOCKS = [(0, 22), (22, 44), (44, 66)]

    with tile.TileContext(nc) as tc:
        with (
            tc.tile_pool(name="persist", bufs=1) as persist,
            tc.tile_pool(name="psum", bufs=8, space="PSUM") as psum_pool,
            tc.tile_pool(name="obuf", bufs=4) as out_pool,
        ):
            w_sb = persist.tile([128, KH * KW * CH * CH * 128], mybir.dt.bfloat16, tag="w")
            x_sb = {}
            for t in range(T):
                for c in range(CH):
                    x_sb[(t, c)] = persist.tile(
                        [128, HP * WP],
                        mybir.dt.bfloat16,
                        name=f"x{t}{c}",
                        tag=f"x{t}{c}",
                    )

            # PE warmup: ~13 throwaway matmuls on zeroed scratch during the
            # DMA prelude, so the HAM clock gate is at 8/8 when the real
            # stream starts (saves ~2us of cold-rate matmuls)
            warm_x = persist.tile([128, 512], mybir.dt.bfloat16, name="warm", tag="warm")
            warm_ps = psum_pool.tile(
                [128, 512], mybir.dt.float32, name="wps", tag="wps", bufs=1
            )
            nc.gpsimd.memset(warm_x[:], 0.0)
            for _ in range(13):
                nc.tensor.matmul(
                    warm_ps[:], warm_x[:, :128], warm_x[:], start=True, stop=True
                )

            NINE = KH * KW * 128  # one (coc, cic) group of 9 weight tiles
            # first-needed slivers first: 9 tiles of (coc0,cic0), x0 rows 0-10,
            # then the rest in need order
            nc.sync.dma_start(w_sb[:, :NINE], w_d[:, :NINE])
            nc.sync.dma_start(x_sb[(0, 0)][:, : 11 * WP], x_d[0, 0, :, : 11 * WP])
            nc.sync.dma_start(w_sb[:, NINE : 2 * NINE], w_d[:, NINE : 2 * NINE])
            nc.sync.dma_start(x_sb[(0, 1)][:, : 11 * WP], x_d[0, 1, :, : 11 * WP])
            nc.sync.dma_start(w_sb[:, 2 * NINE :], w_d[:, 2 * NINE :])
            for c in range(CH):
                nc.sync.dma_start(
                    x_sb[(0, c)][:, 11 * WP : 22 * WP], x_d[0, c, :, 11 * WP : 22 * WP]
                )
            for t in range(T):
                for r0, r1 in ROW_BLOCKS:
                    if t == 0 and r0 == 0:
                        continue
                    for c in range(CH):
                        nc.sync.dma_start(
                            x_sb[(t, c)][:, r0 * WP : r1 * WP],
                            x_d[t, c, :, r0 * WP : r1 * WP],
                        )

            NK = CH * KH * KW  # 18 accumulation steps per output tile
            for t in range(T):
                x3 = {
                    c: x_sb[(t, c)][:].rearrange("p (h w) -> p h w", w=WP)
                    for c in range(CH)
                }
                for coc in range(CH):
                    for yt in range(NYT):
                        ps = psum_pool.tile(
                            [128, ROWS_PER_TILE * W], mybir.dt.float32, bufs=7
                        )
                        y0 = yt * ROWS_PER_TILE
                        for k in range(NK):
                            cic, ky, kx = k // 9, (k % 9) // KW, k % KW
                            idx = (coc * CH + cic) * KH * KW + ky * KW + kx
                            nc.tensor.matmul(
                                ps[:],
                                w_sb[:, idx * 128 : (idx + 1) * 128],
                                x3[cic][
                                    :, y0 + ky : y0 + ky + ROWS_PER_TILE, kx : kx + W
                                ],
                                start=(k == 0),
                                stop=(k == NK - 1),
                            )
                        ob = out_pool.tile([128, ROWS_PER_TILE * W], mybir.dt.float32)
                        nc.vector.tensor_copy(ob[:], ps[:])
                        nc.scalar.dma_start(
                            o_d[t, coc, :, y0 * W : (y0 + ROWS_PER_TILE) * W], ob[:]
                        )

    nc.compile()
    return nc


def _prep_inputs(inputs, ada_weight):
    """Host-side layout prep: pad+cast inputs, transpose weights to lhsT tiles."""
    bf16 = ml_dtypes.bfloat16
    in_maps = []
    for b in range(B):
        xb = inputs[b * T : (b + 1) * T].reshape(T, CH, 128, H, W)
        xp = np.zeros((T, CH, 128, HP, WP), dtype=bf16)
        xp[..., 1 : H + 1, 1 : W + 1] = xb.astype(bf16)

        # ada_weight[b]: [co, ci, ky, kx] -> lhsT tiles [ci, (coc cic ky kx co)]
        wb = ada_weight[b].reshape(CH, 128, CH, 128, KH, KW)  # coc co cic ci ky kx
        wprep = wb.transpose(3, 0, 2, 4, 5, 1)  # ci coc cic ky kx co
        wprep = np.ascontiguousarray(wprep.astype(bf16)).reshape(
            128, KH * KW * CH * CH * 128
        )
        in_maps.append({"x": xp.reshape(T, CH, 128, HP * WP), "w": wprep})
    return in_maps


def _setup_profiling():
    """Register the NTFF profile hook that the image's antenv stub lacks,
    and keep profiling artifacts local. Only used when profile=True."""
    import sys
    import types

    try:
        from antenv.axon_hooks import get_axon_ntff_profile_hook  # noqa: F401

        return
    except ImportError:
        pass
    import antenv
    from trn_agent_boot.trn_boot import _ntff_profile_via_ctypes

    hook = _ntff_profile_via_ctypes("/opt/axon/libaxon_pjrt.so")
    m = types.ModuleType("antenv.axon_hooks")
    m.get_axon_ntff_profile_hook = lambda: hook
    m.set_axon_ntff_profile_hook = lambda h: None
    sys.modules["antenv.axon_hooks"] = m
    antenv.axon_hooks = m

    from concourse import bass_utils

    bass_utils.upload_artifacts = lambda tmpdir: f"file://{tmpdir}"


def kernel(inputs, ada_weight, profile=False, trace_kwargs=None):
    global LAST_EXEC_TIME_NS, LAST_PROFILE
    from concourse.bass_utils import run_bass_kernel_spmd

    if profile:
        _setup_profiling()
    if "nc" not in _cache:
        _cache["nc"] = _build()
    nc = _cache["nc"]

    in_maps = _prep_inputs(np.asarray(inputs), np.asarray(ada_weight))

    kwargs = {}
    if profile:
        kwargs["trace"] = True
        if trace_kwargs:
            kwargs.update(trace_kwargs)
    res = run_bass_kernel_spmd(nc, in_maps, core_ids=list(range(NCORES)), **kwargs)
    if profile:
        LAST_EXEC_TIME_NS = res.exec_time_ns
        LAST_PROFILE = res

    out = np.stack([res.results[b]["out"].reshape(T, COUT, H, W) for b in range(B)])
    return np.ascontiguousarray(out.reshape(B * T, COUT, H, W).astype(np.float32))
